# revision 14
# baseline (speedup 1.0000x reference)
"""AdaptivePoolingAttention on 8 TRN2 NeuronCores — reordered attention.

Data-parallel over segments (4 whole segments per core, attention is
block-diagonal per segment); weights replicated; no collectives.

Key reordering: R=64 queries << L=512 keys per segment, so K and V are
never materialized.  With q' = (q @ wk^T):
  scores^T = x-chunks^T-contracted with q'^T       [contract D]
  p^T      = x-chunks contracted with softmax^T    [contract tokens]
  o^T      = wv-chunks contracted with p^T         [contract D]
This cuts per-core PE work ~4.8M -> ~3.3M cycles and removes the K/V
DRAM spill and all attention-side transposes (scores come out already
token-major; softmax reduces over partitions via a ones-matmul that
also broadcasts the sums to every partition).

Pooling runs on the PE too: block-diagonal 1/8 masks as stationary
operands, 32-row partition-offset accumulation groups.

The main loop is software-pipelined at emission: fold heads interleave
with pT chunks, o-fold heads interleave with the next iteration's
scores chunks, so each DMA stream has 2x the arrival window and the PE
never idles long enough to trip the HAM re-throttle.

Host wrapper pre-packs layouts (pure data movement + bf16 rounding):
  xT       [D, TOK]            x transposed (scores chunks)
  xN_pack  [SEGC, 8, 128, 2048] x natural, swizzled per (seg, dd-quad)
  pool_mask [2, 128, 32]       block-diag mean-pool masks (even/odd)
  wq_pack  [H, 128, D]         wq_pack[h, p, dd*128+c] = wq[dd*128+p, h*128+c]
  wkT_pack [H, 128, D]         wkT_pack[h, p, d]       = wk[d, h*128+p]
  wvh_pack [H, 128, D]         wvh_pack[h, p, dd*128+c] = wv[dd*128+p, h*128+c]
  wo_pack  [D/512, 128, H*512] wo_pack[j, p, hh*512+c] = wo[hh*128+p, j*512+c]
"""

import sys

sys.path.insert(0, "/opt/trn_rl_repo")

import numpy as np
from contextlib import ExitStack

SEG, L, R, POOL, H, HD, D, EPS = 32, 512, 64, 8, 32, 128, 4096, 1e-5
NCORES = 8
SEGC = SEG // NCORES          # 4 segments per core
TOK = SEGC * L                # 2048 tokens per core
Q = SEGC * R                  # 256 queries per core
ND = D // 128                 # 32 contraction chunks
G = 4                         # head groups
HG = H // G                   # 8 heads per group
NJ = 512                      # wo col-tile width

_CACHE = {}


def _build():
    import concourse.bass as bass
    import concourse.mybir as mybir
    import concourse.tile as tile
    from concourse import bacc

    f32 = mybir.dt.float32
    bf16 = mybir.dt.bfloat16
    ts = bass.ts
    ds = bass.ds
    AF = mybir.ActivationFunctionType
    ALU = mybir.AluOpType

    nc = bacc.Bacc("TRN2", target_bir_lowering=False, debug=False)

    xT_e = nc.declare_dram_parameter("xT", [D, TOK], bf16, isOutput=False)
    xN_e = nc.declare_dram_parameter(
        "xN_pack", [SEGC, 8, 128, 2048], bf16, isOutput=False
    )
    pm_e = nc.declare_dram_parameter("pool_mask", [2, 128, 32], bf16, isOutput=False)
    wq_e = nc.declare_dram_parameter("wq_pack", [H, 128, D], bf16, isOutput=False)
    wk_e = nc.declare_dram_parameter("wkT_pack", [H, 128, D], bf16, isOutput=False)
    wv_e = nc.declare_dram_parameter("wvh_pack", [H, 128, D], bf16, isOutput=False)
    wo_e = nc.declare_dram_parameter(
        "wo_pack", [D // NJ, 128, H * NJ], bf16, isOutput=False
    )
    anw_e = nc.declare_dram_parameter("attn_norm_w", [128, D], bf16, isOutput=False)
    onw_e = nc.declare_dram_parameter("out_norm_w", [128, D], bf16, isOutput=False)
    id_e = nc.declare_dram_parameter("ident", [128, 128], bf16, isOutput=False)
    out_e = nc.declare_dram_parameter("out", [Q, D], f32, isOutput=True)

    qspill_d = nc.dram_tensor("q_spill", [2, 128, D], bf16)
    oT_d = nc.dram_tensor("oT_spill", [128, H - HG, Q], bf16)

    with tile.TileContext(nc, pool_alloc_mode="queue") as tc, ExitStack() as st:
        # ---- constants ------------------------------------------------
        cst = st.enter_context(tc.tile_pool(name="const", bufs=1))
        ident = cst.tile([128, 128], bf16)
        nc.sync.dma_start(ident[:], id_e[:])
        masks = [
            cst.tile([128, 32], bf16, tag=f"pm{m}", name=f"pm{m}")
            for m in range(2)
        ]
        for m in range(2):
            nc.sync.dma_start(masks[m][:], pm_e[m])
        epst = cst.tile([128, 1], f32)
        nc.vector.memset(epst[:], EPS)
        ones = cst.tile([128, 128], bf16)
        nc.vector.memset(ones[:], 1.0)

        # qT persists stage C .. last fold; otk holds group-3 oT into F
        qTp = st.enter_context(tc.tile_pool(name="qTp", bufs=1))
        qT = qTp.tile([128, H, Q], bf16)
        otkp = st.enter_context(tc.tile_pool(name="otk", bufs=1))
        otk = otkp.tile([128, HG, Q], bf16)

        # ---- stages A+B: PE-pooled queries -> rmsnorm -> qnT ----------
        with tc.tile_pool(name="qnTp", bufs=1) as qnT_p:
            qnT = qnT_p.tile([128, ND, Q], bf16)
            with tc.tile_pool(name="qn", bufs=1) as qn_p:
                q_nat = [
                    qn_p.tile([128, D], bf16, tag=f"qnat{i}", name=f"qnat{i}")
                    for i in range(2)
                ]
                with (
                    tc.tile_pool(name="xaq", bufs=3) as xaq_p,
                    tc.tile_pool(name="apsA", bufs=8, space="PSUM") as apsA,
                ):
                    qps = {}
                    for qt in range(2):
                        for i in range(8):
                            qps[qt, i] = apsA.tile(
                                [128, 512], f32, tag="qps", name=f"qps{qt}_{i}"
                            )
                    chunks = [
                        (qt, sl, ddq)
                        for qt in range(2)
                        for sl in range(2)
                        for ddq in range(8)
                    ]
                    xaqs = {}

                    def a_load(k):
                        qt, sl, ddq = chunks[k]
                        xaq = xaq_p.tile([128, 2048], bf16, tag="xaq",
                                         name=f"xaq{k}")
                        dmae = nc.sync if k % 2 else nc.scalar
                        dmae.dma_start(xaq[:], xN_e[qt * 2 + sl, ddq])
                        xaqs[k] = xaq

                    a_load(0)
                    a_load(1)
                    for k, (qt, sl, ddq) in enumerate(chunks):
                        if k + 2 < len(chunks):
                            a_load(k + 2)
                        xaq = xaqs.pop(k)
                        for tbl in range(4):
                            gtb = sl * 4 + tbl
                            nc.tensor.matmul(
                                qps[qt, ddq][ds((gtb // 2) * 32, 32), :],
                                masks[gtb % 2][:],
                                xaq[:, ts(tbl, 512)],
                                start=(gtb % 2 == 0),
                                stop=(gtb % 2 == 1),
                                tile_position=(0, (gtb // 2) * 32),
                            )
                        if sl == 1 and gtb == 7:
                            nc.vector.tensor_copy(
                                q_nat[qt][:, ts(ddq, 512)], qps[qt, ddq][:]
                            )

                # stage B: rmsnorm(queries) -> qnT; spill raw queries
                with tc.tile_pool(name="aps", bufs=3, space="PSUM") as aps:
                    bw_attn = qn_p.tile([128, D], bf16, tag="bwa")
                    nc.scalar.dma_start(bw_attn[:], anw_e[:])
                    for qt in range(2):
                        nc.scalar.dma_start(qspill_d[qt], q_nat[qt][:])
                        qnn = qn_p.tile([128, D], bf16, tag="qnn")
                        ssq = qn_p.tile([128, 1], f32, tag="ssq")
                        nc.scalar.activation(
                            qnn[:], q_nat[qt][:], AF.Square, accum_out=ssq[:]
                        )
                        srt = qn_p.tile([128, 1], f32, tag="srt")
                        nc.scalar.activation(
                            srt[:], ssq[:], AF.Sqrt, bias=epst[:], scale=1.0 / D
                        )
                        rs = qn_p.tile([128, 1], f32, tag="rs")
                        nc.vector.reciprocal(rs[:], srt[:])
                        nc.vector.tensor_scalar_mul(qnn[:], q_nat[qt][:], rs[:])
                        nc.vector.tensor_tensor(
                            qnn[:], qnn[:], bw_attn[:], op=ALU.mult
                        )
                        for dblk in range(ND):
                            pt = aps.tile([128, 128], bf16, tag="pt2")
                            nc.tensor.transpose(
                                pt[:], qnn[:, ts(dblk, 128)], ident[:]
                            )
                            nc.any.tensor_copy(qnT[:, dblk, ts(qt, 128)], pt[:])

            # ---- stage C: qT = wq-proj(qnT), scale folded -------------
            with (
                tc.tile_pool(name="wqb", bufs=2) as wqb_p,
                tc.tile_pool(name="cps", bufs=2, space="PSUM") as cps,
            ):
                for h in range(H):
                    wqb = wqb_p.tile([128, ND, 128], bf16, tag="wqb")
                    nc.scalar.dma_start(
                        wqb[:], wq_e[h].rearrange("p (dd c) -> p dd c", c=128)
                    )
                    psq = cps.tile([128, Q], f32, tag="psq")
                    for dblk in range(ND):
                        nc.tensor.matmul(
                            psq[:], wqb[:, dblk, :], qnT[:, dblk, :],
                            start=(dblk == 0), stop=(dblk == ND - 1),
                        )
                    nc.scalar.mul(qT[:, h, :], psq[:], float(HD) ** -0.5)

        # ---- main loop: fold -> scores -> softmax -> pT -> o-fold -----
        with (
            tc.tile_pool(name="qp2", bufs=1) as qp2_p,
            tc.tile_pool(name="pt2", bufs=1) as pt2_p,
            tc.tile_pool(name="esb", bufs=2) as esb_p,
            tc.tile_pool(name="ost", bufs=2) as ost_p,
            tc.tile_pool(name="rcp", bufs=2) as rcp_p,
            tc.tile_pool(name="wkc", bufs=3) as wkc_p,
            tc.tile_pool(name="wvc", bufs=3) as wvc_p,
            tc.tile_pool(name="xtc", bufs=2) as xtc_p,
            tc.tile_pool(name="xnc", bufs=2) as xnc_p,
            tc.tile_pool(name="scps", bufs=4, space="PSUM") as scps,
            tc.tile_pool(name="bigps", bufs=3, space="PSUM") as bigps,
            tc.tile_pool(name="otps", bufs=1, space="PSUM") as otps,
        ):
            qpT2 = qp2_p.tile([128, ND, HG * 128], bf16)    # 64 KiB/part
            pT2 = pt2_p.tile([128, ND, 2 * 512], bf16)      # 64 KiB/part

            def units_fold(g, sp):
                """q'T for heads of group g, queries of seg-pair sp;
                one (load, comp) unit per head."""
                state = {}

                def mk(hl):
                    h = g * HG + hl

                    def ld():
                        halves = []
                        for hf in range(2):
                            wkc = wkc_p.tile([128, D // 2], bf16, tag="wkc",
                                             name=f"wkc{hl}_{hf}")
                            nc.scalar.dma_start(
                                wkc[:], wk_e[h][:, ts(hf, D // 2)]
                            )
                            halves.append(wkc)
                        state[hl] = halves

                    def cp():
                        halves = state.pop(hl)
                        for ddq in range(ND // 4):
                            fps = bigps.tile([128, 512], f32, tag="big",
                                             name=f"fps{hl}_{ddq}")
                            for j in range(4):
                                dd = ddq * 4 + j
                                nc.tensor.matmul(
                                    fps[:, ts(j, 128)],
                                    halves[dd // 16][:, ts(dd % 16, 128)],
                                    qT[:, h, ts(sp, 128)],
                                    start=True, stop=True,
                                )
                            nc.vector.tensor_copy(
                                qpT2[:, ds(ddq * 4, 4), ds(hl * 128, 128)],
                                fps[:].rearrange("p (j c) -> p j c", c=128),
                            )
                    return (ld, cp)
                return [mk(hl) for hl in range(HG)]

            def units_scores(s, sc):
                """scores^T accumulation; one (load, comp) unit per dd-quad."""
                si = s % 2
                state = {}

                def mk(ddq):
                    def ld():
                        xtc = xtc_p.tile([128, 4, 512], bf16, tag="xtc",
                                         name=f"xtc{ddq}")
                        nc.sync.dma_start(
                            xtc[:],
                            xT_e[ds(ddq * 512, 512), ts(s, 512)].rearrange(
                                "(d p) t -> p d t", p=128
                            ),
                        )
                        state[ddq] = xtc

                    def cp():
                        xtc = state.pop(ddq)
                        for j in range(4):
                            dd = ddq * 4 + j
                            rhs = qpT2[:, dd, :].rearrange(
                                "p (hl c) -> p hl c", c=128
                            )[:, :, ds(si * 64, 64)]
                            for tb in range(4):
                                nc.tensor.matmul(
                                    sc[tb][:], xtc[:, j, ts(tb, 128)], rhs,
                                    start=(dd == 0), stop=(dd == ND - 1),
                                )
                    return (ld, cp)
                return [mk(ddq) for ddq in range(ND // 4)]

            def unit_sums(sc, out):
                """exp; ones-matmul broadcast column sums; reciprocal;
                normalize in place.  comp-only unit.  sm is allocated at
                build time so the scps ring order matches use order."""
                sm = scps.tile([128, 512], f32, tag="sc", name="sm")

                def cp():
                    esb = esb_p.tile([128, 4, 512], bf16, tag="esb",
                                     name="esb")
                    for tb in range(4):
                        nc.scalar.activation(esb[:, tb, :], sc[tb][:], AF.Exp)
                    for tb in range(4):
                        nc.tensor.matmul(
                            sm[:], ones[:], esb[:, tb, :],
                            start=(tb == 0), stop=(tb == 3),
                        )
                    rcpb = rcp_p.tile([128, 512], f32, tag="rcpb")
                    nc.vector.reciprocal(rcpb[:], sm[:])
                    for tb in range(4):
                        nc.vector.tensor_tensor(
                            esb[:, tb, :], esb[:, tb, :], rcpb[:], op=ALU.mult
                        )
                    out.append(esb)
                return (None, cp)

            def units_pT(s, esb_ref):
                """pT2[:, :, si]; one (load, comp) unit per dd-quad."""
                si = s % 2
                state = {}

                def mk(ddq):
                    def ld():
                        xnc = xnc_p.tile([128, 16, 128], bf16, tag="xnc",
                                         name=f"xnc{ddq}")
                        nc.gpsimd.dma_start(
                            xnc[:],
                            xN_e[s, ddq].rearrange("p (f c) -> p f c", c=128),
                        )
                        state[ddq] = xnc

                    def cp():
                        xnc = state.pop(ddq)
                        esb = esb_ref[0]
                        for j in range(4):
                            dd = ddq * 4 + j
                            pps = bigps.tile([128, 512], f32, tag="big",
                                             name=f"pps{ddq}_{j}")
                            for tb in range(4):
                                nc.tensor.matmul(
                                    pps[:], xnc[:, tb * 4 + j, :],
                                    esb[:, tb, :],
                                    start=(tb == 0), stop=(tb == 3),
                                )
                            nc.vector.tensor_copy(pT2[:, dd, ts(si, 512)], pps[:])
                    return (ld, cp)
                return [mk(ddq) for ddq in range(ND // 4)]

            def units_ofold(g, sp):
                """oT for group g seg-pair sp; one (load, comp) unit per head."""
                state = {}

                def mk(hl):
                    h = g * HG + hl

                    def ld():
                        halves = []
                        for hf in range(2):
                            wvc = wvc_p.tile([128, D // 2], bf16, tag="wvc",
                                             name=f"wvc{hl}_{hf}")
                            nc.gpsimd.dma_start(
                                wvc[:], wv_e[h][:, ts(hf, D // 2)]
                            )
                            halves.append(wvc)
                        state[hl] = halves

                    def cp():
                        halves = state.pop(hl)
                        ops = otps.tile([128, 128], f32, tag="ot",
                                        name=f"ot{hl}")
                        for dd in range(ND):
                            rhs = pT2[:, dd, :].rearrange(
                                "p (si c) -> p si c", c=512
                            )[:, :, ds(hl * 64, 64)]
                            nc.tensor.matmul(
                                ops[:], halves[dd // 16][:, ts(dd % 16, 128)],
                                rhs,
                                start=(dd == 0), stop=(dd == ND - 1),
                            )
                        if g == G - 1:
                            nc.vector.tensor_copy(otk[:, hl, ts(sp, 128)], ops[:])
                        else:
                            osb = ost_p.tile([128, 128], bf16, tag="osb",
                                             name=f"osb{hl}")
                            nc.vector.tensor_copy(osb[:], ops[:])
                            nc.scalar.dma_start(oT_d[:, h, ts(sp, 128)], osb[:])
                    return (ld, cp)
                return [mk(hl) for hl in range(HG)]

            def interleave(ua, ub):
                out = []
                for a, b in zip(ua, ub):
                    out.append(a)
                    out.append(b)
                return out

            def new_sc(pfx):
                return [
                    scps.tile([128, 512], f32, tag="sc", name=f"{pfx}{tb}")
                    for tb in range(4)
                ]

            # Build the full main-loop unit stream, then emit with loads
            # issued one unit ahead of their compute.
            pairs = [(g, sp) for g in range(G) for sp in range(2)]
            stream = []
            stream += units_fold(*pairs[0])
            sc_next = new_sc("scn0_")
            stream += units_scores(0, sc_next)
            for i, (g, sp) in enumerate(pairs):
                s0, s1 = 2 * sp, 2 * sp + 1
                sc0 = sc_next
                sc1 = new_sc(f"sc1i{i}_")
                stream += units_scores(s1, sc1)
                esb0 = []
                stream.append(unit_sums(sc0, esb0))
                pu0 = units_pT(s0, esb0)
                if i + 1 < len(pairs):
                    stream += interleave(units_fold(*pairs[i + 1]), pu0)
                else:
                    stream += pu0
                esb1 = []
                stream.append(unit_sums(sc1, esb1))
                stream += units_pT(s1, esb1)
                ou = units_ofold(g, sp)
                if i + 1 < len(pairs):
                    sc_next = new_sc(f"scni{i}_")
                    stream += interleave(
                        ou, units_scores(2 * pairs[i + 1][1], sc_next)
                    )
                else:
                    stream += ou

            loads = [u[0] for u in stream if u[0] is not None]
            li = 0
            # prime one load ahead
            if li < len(loads):
                loads[li]()
                li += 1
            for ld, cp in stream:
                if ld is not None and li < len(loads):
                    loads[li]()
                    li += 1
                cp()

        # ---- stage F: out = oT' @ wo + queries, final rmsnorm ---------
        with (
            tc.tile_pool(name="wob", bufs=2) as wob_p,
            tc.tile_pool(name="qrl", bufs=1) as qrl_p,
            tc.tile_pool(name="fout", bufs=1) as fout_p,
            tc.tile_pool(name="fsc", bufs=1) as fsc_p,
            tc.tile_pool(name="fps", bufs=2, space="PSUM") as fps,
        ):
            oTf = qrl_p.tile([128, H - HG, Q], bf16, tag="oTf", name="oTf")
            nc.sync.dma_start(oTf[:], oT_d[:])
            q_rl = [
                qrl_p.tile([128, D], bf16, tag=f"qrl{i}", name=f"qrl{i}")
                for i in range(2)
            ]
            for qt in range(2):
                nc.gpsimd.dma_start(q_rl[qt][:], qspill_d[qt])
            bw_out = fsc_p.tile([128, D], bf16, tag="bwo")
            nc.scalar.dma_start(bw_out[:], onw_e[:])
            out_sb = [
                fout_p.tile([128, D], bf16, tag=f"osb{i}", name=f"osb{i}")
                for i in range(2)
            ]
            hh_order = list(range(H - HG, H)) + list(range(H - HG))
            for j in range(D // NJ):
                wob = wob_p.tile([128, H, NJ], bf16, tag="wob", name="wob")
                nc.scalar.dma_start(
                    wob[:], wo_e[j].rearrange("p (hh c) -> p hh c", c=NJ)
                )
                for qt in range(2):
                    ps = fps.tile([128, NJ], f32, tag="fp")
                    for idx, hh in enumerate(hh_order):
                        src_t = (
                            otk[:, hh - (H - HG), ts(qt, 128)]
                            if hh >= H - HG
                            else oTf[:, hh, ts(qt, 128)]
                        )
                        nc.tensor.matmul(
                            ps[:], src_t, wob[:, hh, :],
                            start=(idx == 0), stop=(idx == H - 1),
                        )
                    nc.vector.tensor_tensor(
                        out_sb[qt][:, ts(j, NJ)], ps[:],
                        q_rl[qt][:, ts(j, NJ)], op=ALU.add,
                    )
            for qt in range(2):
                scrh = fsc_p.tile([128, D // 2], bf16, tag="fscr")
                ssq = fsc_p.tile([128, 1], f32, tag="fssq")
                ssqa = fsc_p.tile([128, 1], f32, tag="fssqa")
                for half in range(2):
                    nc.scalar.activation(
                        scrh[:], out_sb[qt][:, ts(half, D // 2)],
                        AF.Square, accum_out=(ssq if half else ssqa)[:],
                    )
                nc.vector.tensor_tensor(ssq[:], ssq[:], ssqa[:], op=ALU.add)
                srt = fsc_p.tile([128, 1], f32, tag="fsrt")
                nc.scalar.activation(
                    srt[:], ssq[:], AF.Sqrt, bias=epst[:], scale=1.0 / D
                )
                rs = fsc_p.tile([128, 1], f32, tag="frs")
                nc.vector.reciprocal(rs[:], srt[:])
                for half in range(2):
                    fin = fsc_p.tile([128, D // 2], f32, tag="ffin")
                    nc.vector.tensor_scalar_mul(
                        fin[:], out_sb[qt][:, ts(half, D // 2)], rs[:]
                    )
                    nc.vector.tensor_tensor(
                        fin[:], fin[:], bw_out[:, ts(half, D // 2)],
                        op=ALU.mult,
                    )
                    nc.sync.dma_start(
                        out_e[ts(qt, 128), ts(half, D // 2)], fin[:]
                    )

    nc.finalize()
    return nc


def _in_maps(inputs):
    import ml_dtypes

    bf = ml_dtypes.bfloat16
    x = np.asarray(inputs["x"], dtype=np.float32)
    wq = np.asarray(inputs["wq"], dtype=np.float32)
    wkv = np.asarray(inputs["wkv"], dtype=np.float32)
    wo = np.asarray(inputs["wo"], dtype=np.float32)
    wk, wv = wkv[:, : H * HD], wkv[:, H * HD :]

    wq_pack = np.ascontiguousarray(
        wq.astype(bf).reshape(ND, 128, H, 128).transpose(2, 1, 0, 3).reshape(
            H, 128, D
        )
    )
    wkT_pack = np.ascontiguousarray(
        wk.astype(bf).reshape(D, H, 128).transpose(1, 2, 0)
    )
    wvh_pack = np.ascontiguousarray(
        wv.astype(bf).reshape(ND, 128, H, 128).transpose(2, 1, 0, 3).reshape(
            H, 128, D
        )
    )
    wo_pack = np.ascontiguousarray(
        wo.astype(bf).reshape(H, 128, D // NJ, NJ).transpose(2, 1, 0, 3).reshape(
            D // NJ, 128, H * NJ
        )
    )
    anw = np.ascontiguousarray(
        np.broadcast_to(
            np.asarray(inputs["attn_norm_w"], dtype=np.float32).reshape(1, D),
            (128, D),
        ).astype(bf)
    )
    onw = np.ascontiguousarray(
        np.broadcast_to(
            np.asarray(inputs["out_norm_w"], dtype=np.float32).reshape(1, D),
            (128, D),
        ).astype(bf)
    )
    ident = np.eye(128, dtype=np.float32).astype(bf)
    # pool masks: even token-block -> query cols 0..15, odd -> 16..31
    pmask = np.zeros((2, 128, 32), dtype=np.float32)
    for p in range(128):
        pmask[0, p, p // POOL] = 1.0 / POOL
        pmask[1, p, 16 + p // POOL] = 1.0 / POOL
    pmask = pmask.astype(bf)

    maps = []
    for i in range(NCORES):
        xc = x[i * TOK : (i + 1) * TOK].astype(bf)     # [TOK, D]
        xT = np.ascontiguousarray(xc.T)                # [D, TOK]
        # xN_pack[s, ddq, p, (tb*4+dsub)*128+c] = xc[s*512+tb*128+p, (ddq*4+dsub)*128+c]
        xN = np.ascontiguousarray(
            xc.reshape(SEGC, 4, 128, ND // 4, 4, 128)
            .transpose(0, 3, 2, 1, 4, 5)
            .reshape(SEGC, ND // 4, 128, 2048)
        )
        maps.append(
            {
                "xT": xT,
                "xN_pack": xN,
                "pool_mask": pmask,
                "wq_pack": wq_pack,
                "wkT_pack": wkT_pack,
                "wvh_pack": wvh_pack,
                "wo_pack": wo_pack,
                "attn_norm_w": anw,
                "out_norm_w": onw,
                "ident": ident,
            }
        )
    return maps


def kernel(**inputs):
    from concourse.bass_utils import run_bass_kernel_spmd

    if "nc" not in _CACHE:
        _CACHE["nc"] = _build()
    nc = _CACHE["nc"]
    res = run_bass_kernel_spmd(nc, _in_maps(inputs), core_ids=list(range(NCORES)))
    out = np.concatenate(
        [res.results[i]["out"] for i in range(NCORES)], axis=0
    ).astype(np.float32)
    return out
